# revision 16
# baseline (speedup 1.0000x reference)
"""Two-NEFF Trainium2 kernel for fused BatchNorm1d(train) -> Linear -> ELU.

  y = ELU( ((x - mean) * gamma.rsqrt(var+eps) + beta) @ W.T )

Same algorithm as kernel.py (BN folded into the linear layer), but the 2KB
cross-core stat reduction is done on the HOST between two NEFF launches
instead of with an on-device collective: measured on HW, an
InstCollectiveCompute followed by xbar DMA-transposes costs ~0.5 ms extra
(each transpose serializes against the collective machinery), while the two
NEFFs run at full speed. The 64 MiB/core bf16 staging stays on-device as
sharded jax arrays between the launches.

  NEFF A (per core): stream x (f32), PE ones-matmul stats (bf16 operands),
      downcast x -> bf16, stage feature-split halves to DRAM outputs.
  host: sum the 8x[1,1024] partial stats (32 KB).
  NEFF C (per core): finalize stats -> scaled W.T + bias, xbar-transposed
      reads of staged bf16 x as lhsT, bf16 matmuls, bias via rank-1 matmul,
      ELU = relu(y) + min(exp(y)-1, 0), stream y (f32).
"""

import functools
import sys

import numpy as np

if "/opt/trn_rl_repo" not in sys.path:
    sys.path.insert(0, "/opt/trn_rl_repo")

N_TOTAL = 1048576
F = 256
NCORES = 8
N_SHARD = N_TOTAL // NCORES
P = 128
RT = 8
EPS = 1e-5


def _bass(ncores):
    from concourse import bacc

    return bacc.Bacc(
        "TRN2", target_bir_lowering=False, debug=False, num_devices=ncores
    )


def build_a(n_shard=N_SHARD, ncores=NCORES, rt=RT, repeat=1):
    """Phase A: stats + bf16 staging. Outputs: xb0, xb1 [n_shard,128] bf16,
    st [1, 1024] f32 = [sum(512) | sumsq(512)] (pairs to be folded)."""
    import concourse.tile as tile
    from concourse import mybir

    f32 = mybir.dt.float32
    bf16 = mybir.dt.bfloat16
    AF = mybir.ActivationFunctionType

    nc = _bass(ncores)
    x = nc.dram_tensor("x", [n_shard, F], f32, kind="ExternalInput").ap()
    xb0 = nc.dram_tensor("xb0", [n_shard, P], bf16, kind="ExternalOutput").ap()
    xb1 = nc.dram_tensor("xb1", [n_shard, P], bf16, kind="ExternalOutput").ap()
    st = nc.dram_tensor("st", [1, 4 * F], f32, kind="ExternalOutput").ap()

    T = n_shard // (P * rt)
    n_slices = rt * F // 512

    with tile.TileContext(nc) as tc:
        with tc.tile_pool(name="wp", bufs=1) as wp:
            ones_col = wp.tile([P, 1], bf16)
            nc.vector.memset(ones_col[:], 1.0)
            for _rep in range(repeat):
                with tc.tile_pool(name="sa", bufs=3) as sa, tc.tile_pool(
                    name="psA", bufs=1, space="PSUM"
                ) as psA:
                    ps_sum = psA.tile([1, 512], f32, tag="pssum")
                    ps_sq = psA.tile([1, 512], f32, tag="pssq")
                    xv = x.rearrange("(t p j) f -> t p j f", p=P, j=rt)
                    xb0v = xb0.rearrange("(t p j) c -> t p j c", p=P, j=rt)
                    xb1v = xb1.rearrange("(t p j) c -> t p j c", p=P, j=rt)
                    n_mm = T * n_slices
                    k = 0
                    for t in range(T):
                        xt = sa.tile([P, rt, F], f32, tag="xt")
                        nc.sync.dma_start(xt[:], xv[t])
                        sq = sa.tile([P, rt, F], bf16, tag="sq")
                        nc.scalar.activation(sq[:], xt[:], AF.Square)
                        xb = sa.tile([P, rt, F], bf16, tag="xb")
                        nc.vector.tensor_copy(xb[:], xt[:])
                        nc.sync.dma_start(xb0v[t], xb[:, :, 0:P])
                        nc.sync.dma_start(xb1v[t], xb[:, :, P:F])
                        for j2 in range(n_slices):
                            first = k == 0
                            last = k == n_mm - 1
                            nc.tensor.matmul(
                                ps_sum[:],
                                ones_col[:],
                                xb[:, 2 * j2 : 2 * j2 + 2, :],
                                start=first,
                                stop=last,
                            )
                            nc.tensor.matmul(
                                ps_sq[:],
                                ones_col[:],
                                sq[:, 2 * j2 : 2 * j2 + 2, :],
                                start=first,
                                stop=last,
                            )
                            k += 1
                    stats = wp.tile([1, 4 * F], f32)
                    nc.vector.tensor_copy(stats[:, 0:512], ps_sum[:])
                    nc.vector.tensor_copy(stats[:, 512:1024], ps_sq[:])
                    nc.sync.dma_start(st, stats[:])
    nc.compile()
    return nc


def build_c(n_shard=N_SHARD, n_total=N_TOTAL, ncores=NCORES, rt=RT, repeat=1):
    """Phase B'+C: finalize stats (from host-reduced input), matmul + ELU."""
    import concourse.tile as tile
    from concourse import mybir

    f32 = mybir.dt.float32
    bf16 = mybir.dt.bfloat16
    AF = mybir.ActivationFunctionType
    OP = mybir.AluOpType

    nc = _bass(ncores)
    xb0 = nc.dram_tensor("xb0", [n_shard, P], bf16, kind="ExternalInput").ap()
    xb1 = nc.dram_tensor("xb1", [n_shard, P], bf16, kind="ExternalInput").ap()
    wt = nc.dram_tensor("wt", [F, F], f32, kind="ExternalInput").ap()
    gb = nc.dram_tensor("gb", [2, F], f32, kind="ExternalInput").ap()
    sg = nc.dram_tensor("sg", [1, 4 * F], f32, kind="ExternalInput").ap()
    y = nc.dram_tensor("y", [n_shard, F], f32, kind="ExternalOutput").ap()

    T = n_shard // (P * rt)

    with tile.TileContext(nc) as tc:
        with tc.tile_pool(name="wp", bufs=1) as wp, tc.tile_pool(
            name="dram", bufs=1, space="DRAM"
        ) as dr:
            ones_row_bf = wp.tile([1, P], bf16)
            nc.vector.memset(ones_row_bf[:], 1.0)
            wt_sb = wp.tile([P, 2, F], f32)
            nc.sync.dma_start(wt_sb[:], wt.rearrange("(c p) f -> p c f", p=P))
            ga_sb = wp.tile([1, F], f32)
            nc.sync.dma_start(ga_sb[:], gb[0:1, :])
            be_sb = wp.tile([1, F], f32)
            nc.sync.dma_start(be_sb[:], gb[1:2, :])

            for _rep in range(repeat):
                with tc.tile_pool(name="pb", bufs=1, space="PSUM") as psB:
                    g = wp.tile([1, 4 * F], f32)
                    nc.sync.dma_start(g[:], sg)
                    fsum = wp.tile([1, F], f32)
                    nc.vector.tensor_add(fsum[:], g[:, 0:F], g[:, F : 2 * F])
                    fsq = wp.tile([1, F], f32)
                    nc.vector.tensor_add(
                        fsq[:], g[:, 2 * F : 3 * F], g[:, 3 * F : 4 * F]
                    )
                    mean = wp.tile([1, F], f32)
                    nc.vector.tensor_scalar_mul(mean[:], fsum[:], 1.0 / n_total)
                    var = wp.tile([1, F], f32)
                    nc.vector.tensor_mul(var[:], mean[:], mean[:])
                    nc.vector.scalar_tensor_tensor(
                        var[:], fsq[:], 1.0 / n_total, var[:], OP.mult, OP.subtract
                    )
                    nc.vector.tensor_scalar_add(var[:], var[:], EPS)
                    inv = wp.tile([1, F], f32)
                    nc.vector.reciprocal(inv[:], var[:])
                    rstd = wp.tile([1, F], f32)
                    nc.scalar.activation(rstd[:], inv[:], AF.Sqrt)
                    srow = wp.tile([1, F], f32)
                    nc.vector.tensor_mul(srow[:], ga_sb[:], rstd[:])
                    trow = wp.tile([1, F], f32)
                    nc.vector.tensor_mul(trow[:], mean[:], srow[:])
                    nc.vector.tensor_sub(trow[:], be_sb[:], trow[:])

                    st_d = dr.tile([2, F], f32)
                    nc.sync.dma_start(st_d[0:1, :], srow[:])
                    nc.sync.dma_start(st_d[1:2, :], trow[:])
                    sT = wp.tile([P, 2], f32)
                    nc.sync.dma_start(
                        sT[:], st_d[0:1, :].rearrange("o (c p) -> p (o c)", p=P)
                    )
                    tT = wp.tile([P, 2], f32)
                    nc.sync.dma_start(
                        tT[:], st_d[1:2, :].rearrange("o (c p) -> p (o c)", p=P)
                    )
                    wts = wp.tile([P, 2, F], bf16)
                    for c in range(2):
                        nc.vector.tensor_scalar(
                            wts[:, c, :],
                            wt_sb[:, c, :],
                            sT[:, c : c + 1],
                            None,
                            OP.mult,
                        )
                    ps_b = psB.tile([1, F], f32, tag="psb")
                    for c in range(2):
                        nc.tensor.matmul(
                            ps_b[:],
                            tT[:, c : c + 1],
                            wt_sb[:, c, :],
                            start=(c == 0),
                            stop=(c == 1),
                        )
                    b_bf = wp.tile([1, F], bf16)
                    nc.vector.tensor_copy(b_bf[:], ps_b[:])

                with tc.tile_pool(name="cp", bufs=3) as cp, tc.tile_pool(
                    name="psC", bufs=2, space="PSUM"
                ) as psC:
                    yv = y.rearrange("(t rb q) f -> t q rb f", q=P, rb=rt)
                    R = P * rt
                    for t in range(T):
                        xT0 = cp.tile([P, R], bf16, tag="x0")
                        nc.sync.dma_start_transpose(
                            xT0[:], xb0[t * R : (t + 1) * R, :]
                        )
                        xT1 = cp.tile([P, R], bf16, tag="x1")
                        nc.sync.dma_start_transpose(
                            xT1[:], xb1[t * R : (t + 1) * R, :]
                        )
                        ps_y = psC.tile([P, rt * F], f32, tag="psy")
                        for rb in range(rt):
                            sl = slice(rb * P, (rb + 1) * P)
                            out_sl = ps_y[:, rb * F : (rb + 1) * F]
                            nc.tensor.matmul(
                                out_sl, xT0[:, sl], wts[:, 0, :], start=True, stop=False
                            )
                            nc.tensor.matmul(
                                out_sl, xT1[:, sl], wts[:, 1, :], start=False, stop=False
                            )
                            nc.tensor.matmul(
                                out_sl, ones_row_bf[:], b_bf[:], start=False, stop=True
                            )
                        e = cp.tile([P, rt * F], f32, tag="e")
                        nc.scalar.activation(e[:], ps_y[:], AF.Exp)
                        em = cp.tile([P, rt * F], f32, tag="em")
                        nc.vector.tensor_scalar(
                            em[:], e[:], 1.0, 0.0, OP.subtract, OP.min
                        )
                        yo = cp.tile([P, rt * F], f32, tag="yo")
                        nc.vector.scalar_tensor_tensor(
                            yo[:], ps_y[:], 0.0, em[:], OP.max, OP.add
                        )
                        nc.sync.dma_start(
                            yv[t], yo[:].rearrange("q (rb f) -> q rb f", f=F)
                        )
    nc.compile()
    return nc


@functools.lru_cache(maxsize=4)
def _built_a(repeat=1):
    return build_a(repeat=repeat)


@functools.lru_cache(maxsize=4)
def _built_c(repeat=1):
    return build_c(repeat=repeat)


def _pjrt_fn(nc, ncores=NCORES):
    """Compile a bass module into a jitted 8-core shard_map callable.
    Returns (fn, in_names, out_names, out_avals)."""
    import jax
    from jax.experimental.shard_map import shard_map
    from jax.sharding import Mesh, PartitionSpec

    from concourse import mybir
    from concourse.bass2jax import (
        _bass_exec_p,
        install_neuronx_cc_hook,
        partition_id_tensor,
    )

    install_neuronx_cc_hook()
    partition_name = nc.partition_id_tensor.name if nc.partition_id_tensor else None
    in_names, out_names, out_avals = [], [], []
    for alloc in nc.m.functions[0].allocations:
        if not isinstance(alloc, mybir.MemoryLocationSet):
            continue
        name = alloc.memorylocations[0].name
        if alloc.kind == "ExternalInput":
            if name != partition_name:
                in_names.append(name)
        elif alloc.kind == "ExternalOutput":
            out_names.append(name)
            out_avals.append(
                jax.core.ShapedArray(
                    tuple(alloc.tensor_shape), mybir.dt.np(alloc.dtype)
                )
            )
    n_params = len(in_names)
    all_in_names = list(in_names) + list(out_names)
    if partition_name is not None:
        all_in_names.append(partition_name)

    def _body(*args):
        operands = list(args)
        if partition_name is not None:
            operands.append(partition_id_tensor())
        outs = _bass_exec_p.bind(
            *operands,
            out_avals=tuple(out_avals),
            in_names=tuple(all_in_names),
            out_names=tuple(out_names),
            lowering_input_output_aliases=(),
            sim_require_finite=True,
            sim_require_nnan=True,
            nc=nc,
        )
        return tuple(outs)

    devices = jax.devices()[:ncores]
    mesh = Mesh(np.asarray(devices), ("core",))
    spec = PartitionSpec("core")
    fn = jax.jit(
        shard_map(
            _body,
            mesh=mesh,
            in_specs=(spec,) * (n_params + len(out_names)),
            out_specs=(spec,) * len(out_names),
            check_rep=False,
        ),
        keep_unused=True,
    )
    return fn, in_names, out_names, out_avals, mesh


def _sharding():
    import jax
    from jax.sharding import Mesh, NamedSharding, PartitionSpec

    devices = jax.devices()[:NCORES]
    mesh = Mesh(np.asarray(devices), ("core",))
    return NamedSharding(mesh, PartitionSpec("core"))


def _zeros_for(out_avals):
    return [
        np.zeros((NCORES * av.shape[0], *av.shape[1:]), av.dtype) for av in out_avals
    ]


def kernel(x, gamma, beta, W):
    import jax

    x = np.ascontiguousarray(np.asarray(x), dtype=np.float32)
    gamma = np.asarray(gamma, dtype=np.float32)
    beta = np.asarray(beta, dtype=np.float32)
    W = np.asarray(W, dtype=np.float32)
    assert x.shape == (N_TOTAL, F), x.shape

    sharding = _sharding()

    # ---- NEFF A: stats + staging
    nc_a = _built_a()
    fn_a, in_a, out_a, av_a, _ = _pjrt_fn(nc_a)
    assert in_a == ["x"], in_a
    x_dev = jax.device_put(x, sharding)
    outs_a = fn_a(x_dev, *[jax.device_put(z, sharding) for z in _zeros_for(av_a)])
    outs_a = dict(zip(out_a, outs_a))

    # ---- host: reduce the 8 partial stat rows (32 KB total)
    st_host = np.asarray(outs_a["st"])  # [8*1, 1024]
    stats_g = st_host.reshape(NCORES, 4 * F).sum(axis=0, dtype=np.float64)
    stats_g = np.ascontiguousarray(
        np.broadcast_to(stats_g.astype(np.float32), (NCORES, 4 * F))
    )

    # ---- NEFF C: matmul + ELU (staging stays on device)
    nc_c = _built_c()
    fn_c, in_c, out_c, av_c, _ = _pjrt_fn(nc_c)
    wt_host = np.ascontiguousarray(W.T)
    gb_host = np.stack([gamma, beta])
    host_ins = {
        "wt": np.concatenate([wt_host] * NCORES, axis=0),
        "gb": np.concatenate([gb_host] * NCORES, axis=0),
        "sg": stats_g,
    }
    args_c = []
    for nm in in_c:
        if nm in ("xb0", "xb1"):
            args_c.append(outs_a[nm])
        else:
            args_c.append(jax.device_put(host_ins[nm], sharding))
    outs_c = fn_c(*args_c, *[jax.device_put(z, sharding) for z in _zeros_for(av_c)])
    y = np.asarray(outs_c[out_c.index("y")])
    return np.ascontiguousarray(y.reshape(N_TOTAL, F), dtype=np.float32)


if __name__ == "__main__":
    nca = build_a()
    ncc = build_c()
    print("built OK")
